# revision 24
# baseline (speedup 1.0000x reference)
"""Equivariant LayerNorm (128x0e + 64x1e + 32x2e irreps) on 8 Trainium2 cores.

Input : node_input [200000, 480] f32, affine_weight [224] f32, affine_bias [128] f32
Output: [200000, 480] f32

Feature layout per node:
  block0 cols [0,128)   : 128 scalars (l=0)  -> LayerNorm over the 128 channels,
                          then *w[c] + b[c]
  block1 cols [128,320) : 64 muls x d=3      -> x * w[128+c//3] / sqrt(mean_sq + eps)
  block2 cols [320,480) : 32 muls x d=5      -> x * w[192+c//5] / sqrt(mean_sq + eps)

Sharding: pure data-parallel over nodes: 8 cores x 25000 rows. The tiny affine
params are expanded/broadcast host-side and replicated to every core.

Engine split per 128-row group:
  DVE  : bn_stats/bn_aggr (block0 mean/var), reciprocal, fused
         scalar_tensor_tensor applies for block0 and block1
  ACT  : block1/2 sum-of-squares (Square activation with accum_out),
         sqrt(var + eps) with the 1/(mul*d) scale folded in
  Pool : block2 apply as two TensorTensor ops (broadcast inv2)
  SP   : HWDGE DMAs

This walrus build encodes at most ~1 sync wait per instruction, so after Tile
schedules the program we hoist excess waits onto standalone EventSemaphore
instructions (see _split_excess_waits).
"""

import numpy as np

import concourse.bass as bass
import concourse.mybir as mybir
import concourse.tile as tile
from concourse.bass_utils import run_bass_kernel_spmd

F32 = mybir.dt.float32
EPS = 1e-5
FEAT = 480
N_NODES = 200000
N_CORES = 8
ROWS_PER_CORE = N_NODES // N_CORES  # 25000

# feature blocks: (col_start, col_end, mul, d)
BLOCKS = [(0, 128, 128, 1), (128, 320, 64, 3), (320, 480, 32, 5)]

GROUP_ROWS = 128   # rows per group (SBUF partitions)
G_SUPER = 5        # groups per supertile

AF = mybir.ActivationFunctionType
ALU = mybir.AluOpType


def _chunks(rows):
    """(row_start, n_partitions, n_groups) supertile chunks covering rows."""
    out = []
    r = 0
    super_rows = GROUP_ROWS * G_SUPER
    while rows - r >= super_rows:
        out.append((r, GROUP_ROWS, G_SUPER))
        r += super_rows
    while rows - r >= GROUP_ROWS:
        out.append((r, GROUP_ROWS, 1))
        r += GROUP_ROWS
    if rows - r > 0:
        out.append((r, rows - r, 1))
    return out


def _split_excess_waits(nc, max_waits=1):
    """Hoist waits beyond `max_waits` onto standalone same-engine
    EventSemaphore instructions placed just before the owner.

    This walrus build encodes very few sync commands per instruction; a bare
    EventSemaphore wait on the same sequencer is semantically identical
    (waits are monotonic and execute in sequencer order).
    """
    n = 0
    for bb in nc.main_func.blocks:
        insts = bb.instructions
        out = []
        for inst in insts:
            si = getattr(inst, "sync_info", None)
            waits = list(si.on_wait) if si is not None and si.on_wait else []
            if len(waits) > max_waits:
                for w in waits[:-max_waits]:
                    n += 1
                    ev = mybir.InstEventSemaphore(
                        name=f"EVW-{n}-{inst.name}", ins=[], outs=[]
                    )
                    ev.engine = inst.engine
                    ev.sync_info = mybir.SyncInfo(on_wait=[w], on_update=[])
                    nc.register_instruction(ev, overwrite=True)
                    out.append(ev)
                inst.sync_info = mybir.SyncInfo(
                    on_wait=waits[-max_waits:], on_update=list(si.on_update)
                )
            out.append(inst)
        insts.clear()
        insts.extend(out)


def build_nc(rows=ROWS_PER_CORE):
    nc = bass.Bass("TRN2", target_bir_lowering=False, debug=False)
    x = nc.dram_tensor("x", [rows, FEAT], F32, kind="ExternalInput")
    wt = nc.dram_tensor("wt", [128, FEAT], F32, kind="ExternalInput")
    bt = nc.dram_tensor("bt", [128, 128], F32, kind="ExternalInput")
    y = nc.dram_tensor("y", [rows, FEAT], F32, kind="ExternalOutput")

    with tile.TileContext(nc) as tc:
        with (
            tc.tile_pool(name="const", bufs=1) as const,
            tc.tile_pool(name="xin", bufs=4) as xin,
            tc.tile_pool(name="yout", bufs=3) as yout,
            tc.tile_pool(name="stats", bufs=3) as stats,
            tc.tile_pool(name="scr", bufs=2) as scr,
            tc.tile_pool(name="t0p", bufs=G_SUPER + 1) as t0p,
        ):
            W = const.tile([128, FEAT], F32)
            nc.sync.dma_start(W[:, :], wt[:, :])
            B = const.tile([128, 128], F32)
            nc.sync.dma_start(B[:, :], bt[:, :])
            EPSC = const.tile([128, 1], F32)
            nc.vector.memset(EPSC[:, :], EPS)

            for r0, P, G in _chunks(rows):
                X = xin.tile([128, G_SUPER * FEAT], F32, tag="X")
                xs = x[r0:r0 + G * P, :].rearrange("(g p) c -> p g c", p=P)
                X3 = X[:P, 0:G * FEAT].rearrange("p (g c) -> p g c", g=G)
                nc.sync.dma_start(X3, xs)

                Y = yout.tile([128, G_SUPER * FEAT], F32, tag="Y")
                BN6 = stats.tile([128, G_SUPER * 6], F32, tag="BN6")
                AGG = stats.tile([128, G_SUPER * 2], F32, tag="AGG")
                Q = stats.tile([128, G_SUPER * 2], F32, tag="Q")
                SD = stats.tile([128, G_SUPER * 3], F32, tag="SD")
                INV = stats.tile([128, G_SUPER * 3], F32, tag="INV")
                SCR = scr.tile([128, 352], F32, tag="SCR")
                PT = scr.tile([128, 160], F32, tag="PT")

                # block0 mean/var (DVE)
                for g in range(G):
                    nc.vector.bn_stats(
                        BN6[:P, 6 * g:6 * g + 6], X[:P, g * FEAT:g * FEAT + 128]
                    )
                for g in range(G):
                    nc.vector.bn_aggr(
                        AGG[:P, 2 * g:2 * g + 2], BN6[:P, 6 * g:6 * g + 6]
                    )

                # block1/2 sums of squares (ACT)
                for g in range(G):
                    c0 = g * FEAT
                    nc.scalar.activation(
                        SCR[:P, 0:192], X[:P, c0 + 128:c0 + 320], AF.Square,
                        accum_out=Q[:P, 2 * g:2 * g + 1],
                    )
                    nc.scalar.activation(
                        SCR[:P, 192:352], X[:P, c0 + 320:c0 + 480], AF.Square,
                        accum_out=Q[:P, 2 * g + 1:2 * g + 2],
                    )

                # block0 apply part 1 (DVE): T0 = (x0 - mean) * w0
                T0s = []
                for g in range(G):
                    c0 = g * FEAT
                    mean = AGG[:P, 2 * g:2 * g + 1]
                    T0 = t0p.tile([128, 128], F32, tag="T0")
                    T0s.append(T0)
                    nc.vector.scalar_tensor_tensor(
                        T0[:P, :], X[:P, c0:c0 + 128], mean, W[:P, 0:128],
                        op0=ALU.subtract, op1=ALU.mult,
                    )

                # sd = sqrt(var + eps); sqrt(q/(mul*d) + eps)  (ACT)
                AGG3 = AGG[:P, 0:2 * G].rearrange("p (g k) -> p g k", g=G)
                Q3 = Q[:P, 0:2 * G].rearrange("p (g k) -> p g k", g=G)
                SD3 = SD[:P, 0:3 * G].rearrange("p (g k) -> p g k", g=G)
                eps_ap = EPSC[:P, 0:1]
                nc.scalar.activation(SD3[:, :, 0:1], AGG3[:, :, 1:2], AF.Sqrt,
                                     bias=eps_ap)
                nc.scalar.activation(SD3[:, :, 1:2], Q3[:, :, 0:1], AF.Sqrt,
                                     bias=eps_ap, scale=1.0 / 192)
                nc.scalar.activation(SD3[:, :, 2:3], Q3[:, :, 1:2], AF.Sqrt,
                                     bias=eps_ap, scale=1.0 / 160)

                nc.vector.reciprocal(INV[:P, 0:3 * G], SD[:P, 0:3 * G])

                for g in range(G):
                    c0 = g * FEAT
                    rstd = INV[:P, 3 * g:3 * g + 1]
                    inv1 = INV[:P, 3 * g + 1:3 * g + 2]
                    inv2 = INV[:P, 3 * g + 2:3 * g + 3]
                    # block0 apply part 2 (DVE): y0 = t0 * rstd + b
                    nc.vector.scalar_tensor_tensor(
                        Y[:P, c0:c0 + 128], T0s[g][:P, :], rstd, B[:P, :],
                        op0=ALU.mult, op1=ALU.add,
                    )
                    # block1 apply (DVE): y1 = (x1 * inv1) * w1
                    nc.vector.scalar_tensor_tensor(
                        Y[:P, c0 + 128:c0 + 320], X[:P, c0 + 128:c0 + 320],
                        inv1, W[:P, 128:320], op0=ALU.mult, op1=ALU.mult,
                    )
                    # block2 apply (Pool): y2 = (x2 * inv2bcast) * w2
                    nc.gpsimd.tensor_tensor(
                        PT[:P, :], X[:P, c0 + 320:c0 + 480],
                        inv2.to_broadcast((P, 160)), op=ALU.mult,
                    )
                    nc.gpsimd.tensor_tensor(
                        Y[:P, c0 + 320:c0 + 480], PT[:P, :], W[:P, 320:480],
                        op=ALU.mult,
                    )

                ys = y[r0:r0 + G * P, :].rearrange("(g p) c -> p g c", p=P)
                Y3 = Y[:P, 0:G * FEAT].rearrange("p (g c) -> p g c", g=G)
                nc.sync.dma_start(ys, Y3)

    _split_excess_waits(nc)
    return nc


def _expand_params(affine_weight, affine_bias):
    w = np.asarray(affine_weight, dtype=np.float32)
    b = np.asarray(affine_bias, dtype=np.float32)
    parts = []
    iw = 0
    for _, _, mul, d in BLOCKS:
        parts.append(np.repeat(w[iw:iw + mul], d))
        iw += mul
    wexp = np.concatenate(parts)  # [480]
    wt = np.tile(wexp[None, :], (128, 1)).astype(np.float32)
    bt = np.tile(b[None, :], (128, 1)).astype(np.float32)
    return wt, bt


_NC_CACHE = {}


def _get_nc(rows):
    if rows not in _NC_CACHE:
        _NC_CACHE[rows] = build_nc(rows)
    return _NC_CACHE[rows]


PROFILE = False
LAST_RESULTS = None


def kernel(node_input, affine_weight, affine_bias):
    global LAST_RESULTS
    x = np.ascontiguousarray(np.asarray(node_input, dtype=np.float32))
    wt, bt = _expand_params(affine_weight, affine_bias)
    nc = _get_nc(ROWS_PER_CORE)
    shards = x.reshape(N_CORES, ROWS_PER_CORE, FEAT)
    in_maps = [
        {"x": shards[i], "wt": wt, "bt": bt} for i in range(N_CORES)
    ]
    res = run_bass_kernel_spmd(
        nc, in_maps, core_ids=list(range(N_CORES)), trace=PROFILE
    )
    LAST_RESULTS = res
    return np.concatenate([r["y"] for r in res.results], axis=0)


# revision 26
# speedup vs baseline: 8.0015x; 8.0015x over previous
"""Equivariant LayerNorm (128x0e + 64x1e + 32x2e irreps) on 8 Trainium2 cores.

Input : node_input [200000, 480] f32, affine_weight [224] f32, affine_bias [128] f32
Output: [200000, 480] f32

Feature layout per node:
  block0 cols [0,128)   : 128 scalars (l=0)  -> LayerNorm over the 128 channels,
                          then *w[c] + b[c]
  block1 cols [128,320) : 64 muls x d=3      -> x * w[128+c//3] / sqrt(mean_sq + eps)
  block2 cols [320,480) : 32 muls x d=5      -> x * w[192+c//5] / sqrt(mean_sq + eps)

Sharding: pure data-parallel over nodes: 8 cores x 25000 rows. The tiny affine
params are expanded/broadcast host-side and replicated to every core.

Engine split per 128-row group:
  DVE  : bn_stats/bn_aggr (block0 mean/var), reciprocal, fused
         scalar_tensor_tensor applies for block0 and block1
  ACT  : block1/2 sum-of-squares (Square activation with accum_out),
         sqrt(var + eps) with the 1/(mul*d) scale folded in
  Pool : block2 apply as two TensorTensor ops (broadcast inv2)
  SP   : HWDGE DMAs

This walrus build encodes at most ~1 sync wait per instruction, so after Tile
schedules the program we hoist excess waits onto standalone EventSemaphore
instructions (see _split_excess_waits).
"""

import numpy as np

import concourse.bass as bass
import concourse.mybir as mybir
import concourse.tile as tile
from concourse.bass_utils import run_bass_kernel_spmd

F32 = mybir.dt.float32
EPS = 1e-5
FEAT = 480
N_NODES = 200000
N_CORES = 8
ROWS_PER_CORE = N_NODES // N_CORES  # 25000

# feature blocks: (col_start, col_end, mul, d)
BLOCKS = [(0, 128, 128, 1), (128, 320, 64, 3), (320, 480, 32, 5)]

GROUP_ROWS = 128   # rows per group (SBUF partitions)
G_SUPER = 8        # groups per supertile

AF = mybir.ActivationFunctionType
ALU = mybir.AluOpType


def _chunks(rows):
    """(row_start, n_partitions, n_groups) supertile chunks covering rows."""
    out = []
    r = 0
    super_rows = GROUP_ROWS * G_SUPER
    while rows - r >= super_rows:
        out.append((r, GROUP_ROWS, G_SUPER))
        r += super_rows
    while rows - r >= GROUP_ROWS:
        out.append((r, GROUP_ROWS, 1))
        r += GROUP_ROWS
    if rows - r > 0:
        out.append((r, rows - r, 1))
    return out


def _split_excess_waits(nc, max_waits=1):
    """Hoist waits beyond `max_waits` onto standalone same-engine
    EventSemaphore instructions placed just before the owner.

    This walrus build encodes very few sync commands per instruction; a bare
    EventSemaphore wait on the same sequencer is semantically identical
    (waits are monotonic and execute in sequencer order).
    """
    n = 0
    for bb in nc.main_func.blocks:
        insts = bb.instructions
        out = []
        for inst in insts:
            si = getattr(inst, "sync_info", None)
            waits = list(si.on_wait) if si is not None and si.on_wait else []
            if len(waits) > max_waits:
                for w in waits[:-max_waits]:
                    n += 1
                    ev = mybir.InstEventSemaphore(
                        name=f"EVW-{n}-{inst.name}", ins=[], outs=[]
                    )
                    ev.engine = inst.engine
                    ev.sync_info = mybir.SyncInfo(on_wait=[w], on_update=[])
                    nc.register_instruction(ev, overwrite=True)
                    out.append(ev)
                inst.sync_info = mybir.SyncInfo(
                    on_wait=waits[-max_waits:], on_update=list(si.on_update)
                )
            out.append(inst)
        insts.clear()
        insts.extend(out)


def build_nc(rows=ROWS_PER_CORE, reps=1):
    nc = bass.Bass("TRN2", target_bir_lowering=False, debug=False)
    x = nc.dram_tensor("x", [rows, FEAT], F32, kind="ExternalInput")
    wt = nc.dram_tensor("wt", [128, FEAT], F32, kind="ExternalInput")
    bt = nc.dram_tensor("bt", [128, 128], F32, kind="ExternalInput")
    y = nc.dram_tensor("y", [rows, FEAT], F32, kind="ExternalOutput")

    with tile.TileContext(nc) as tc:
        with (
            tc.tile_pool(name="const", bufs=1) as const,
            tc.tile_pool(name="xin", bufs=5) as xin,
            tc.tile_pool(name="yout", bufs=3) as yout,
            tc.tile_pool(name="stats", bufs=3) as stats,
            tc.tile_pool(name="scr", bufs=2) as scr,
            tc.tile_pool(name="t0p", bufs=G_SUPER + 1) as t0p,
        ):
            W = const.tile([128, FEAT], F32)
            nc.sync.dma_start(W[:, :], wt[:, :])
            B = const.tile([128, 128], F32)
            nc.sync.dma_start(B[:, :], bt[:, :])
            EPSC = const.tile([128, 1], F32)
            nc.vector.memset(EPSC[:, :], EPS)

            for rep in range(reps):
              for r0, P, G in _chunks(rows):
                X = xin.tile([128, G_SUPER * FEAT], F32, tag="X")
                xs = x[r0:r0 + G * P, :].rearrange("(g p) c -> p g c", p=P)
                X3 = X[:P, 0:G * FEAT].rearrange("p (g c) -> p g c", g=G)
                nc.sync.dma_start(X3, xs)

                Y = yout.tile([128, G_SUPER * FEAT], F32, tag="Y")
                BN6 = stats.tile([128, G_SUPER * 6], F32, tag="BN6")
                AGG = stats.tile([128, G_SUPER * 2], F32, tag="AGG")
                Q = stats.tile([128, G_SUPER * 2], F32, tag="Q")
                SD = stats.tile([128, G_SUPER * 3], F32, tag="SD")
                INV = stats.tile([128, G_SUPER * 3], F32, tag="INV")
                SCR = scr.tile([128, 352], F32, tag="SCR")
                PT = scr.tile([128, 160], F32, tag="PT")

                # block0 mean/var (DVE)
                for g in range(G):
                    nc.vector.bn_stats(
                        BN6[:P, 6 * g:6 * g + 6], X[:P, g * FEAT:g * FEAT + 128]
                    )
                for g in range(G):
                    nc.vector.bn_aggr(
                        AGG[:P, 2 * g:2 * g + 2], BN6[:P, 6 * g:6 * g + 6]
                    )

                # block1/2 sums of squares (ACT)
                for g in range(G):
                    c0 = g * FEAT
                    nc.scalar.activation(
                        SCR[:P, 0:192], X[:P, c0 + 128:c0 + 320], AF.Square,
                        accum_out=Q[:P, 2 * g:2 * g + 1],
                    )
                    nc.scalar.activation(
                        SCR[:P, 192:352], X[:P, c0 + 320:c0 + 480], AF.Square,
                        accum_out=Q[:P, 2 * g + 1:2 * g + 2],
                    )

                # block0 apply part 1 (DVE): T0 = (x0 - mean) * w0
                T0s = []
                for g in range(G):
                    c0 = g * FEAT
                    mean = AGG[:P, 2 * g:2 * g + 1]
                    T0 = t0p.tile([128, 128], F32, tag="T0")
                    T0s.append(T0)
                    nc.vector.scalar_tensor_tensor(
                        T0[:P, :], X[:P, c0:c0 + 128], mean, W[:P, 0:128],
                        op0=ALU.subtract, op1=ALU.mult,
                    )

                # sd = sqrt(var + eps); sqrt(q/(mul*d) + eps)  (ACT)
                AGG3 = AGG[:P, 0:2 * G].rearrange("p (g k) -> p g k", g=G)
                Q3 = Q[:P, 0:2 * G].rearrange("p (g k) -> p g k", g=G)
                SD3 = SD[:P, 0:3 * G].rearrange("p (g k) -> p g k", g=G)
                eps_ap = EPSC[:P, 0:1]
                nc.scalar.activation(SD3[:, :, 0:1], AGG3[:, :, 1:2], AF.Sqrt,
                                     bias=eps_ap)
                nc.scalar.activation(SD3[:, :, 1:2], Q3[:, :, 0:1], AF.Sqrt,
                                     bias=eps_ap, scale=1.0 / 192)
                nc.scalar.activation(SD3[:, :, 2:3], Q3[:, :, 1:2], AF.Sqrt,
                                     bias=eps_ap, scale=1.0 / 160)

                nc.vector.reciprocal(INV[:P, 0:3 * G], SD[:P, 0:3 * G])

                for g in range(G):
                    c0 = g * FEAT
                    rstd = INV[:P, 3 * g:3 * g + 1]
                    inv1 = INV[:P, 3 * g + 1:3 * g + 2]
                    inv2 = INV[:P, 3 * g + 2:3 * g + 3]
                    # block0 apply part 2 (DVE): y0 = t0 * rstd + b
                    nc.vector.scalar_tensor_tensor(
                        Y[:P, c0:c0 + 128], T0s[g][:P, :], rstd, B[:P, :],
                        op0=ALU.mult, op1=ALU.add,
                    )
                    # block1 apply (DVE): y1 = (x1 * inv1) * w1
                    nc.vector.scalar_tensor_tensor(
                        Y[:P, c0 + 128:c0 + 320], X[:P, c0 + 128:c0 + 320],
                        inv1, W[:P, 128:320], op0=ALU.mult, op1=ALU.mult,
                    )
                    # block2 apply (Pool): y2 = (x2 * inv2bcast) * w2
                    nc.gpsimd.tensor_tensor(
                        PT[:P, :], X[:P, c0 + 320:c0 + 480],
                        inv2.to_broadcast((P, 160)), op=ALU.mult,
                    )
                    nc.gpsimd.tensor_tensor(
                        Y[:P, c0 + 320:c0 + 480], PT[:P, :], W[:P, 320:480],
                        op=ALU.mult,
                    )

                ys = y[r0:r0 + G * P, :].rearrange("(g p) c -> p g c", p=P)
                Y3 = Y[:P, 0:G * FEAT].rearrange("p (g c) -> p g c", g=G)
                nc.sync.dma_start(ys, Y3)

    _split_excess_waits(nc)
    return nc


def _expand_params(affine_weight, affine_bias):
    w = np.asarray(affine_weight, dtype=np.float32)
    b = np.asarray(affine_bias, dtype=np.float32)
    parts = []
    iw = 0
    for _, _, mul, d in BLOCKS:
        parts.append(np.repeat(w[iw:iw + mul], d))
        iw += mul
    wexp = np.concatenate(parts)  # [480]
    wt = np.tile(wexp[None, :], (128, 1)).astype(np.float32)
    bt = np.tile(b[None, :], (128, 1)).astype(np.float32)
    return wt, bt


_NC_CACHE = {}


def _get_nc(rows):
    if rows not in _NC_CACHE:
        _NC_CACHE[rows] = build_nc(rows)
    return _NC_CACHE[rows]


PROFILE = False
LAST_RESULTS = None


def kernel(node_input, affine_weight, affine_bias):
    global LAST_RESULTS
    x = np.ascontiguousarray(np.asarray(node_input, dtype=np.float32))
    wt, bt = _expand_params(affine_weight, affine_bias)
    nc = _get_nc(ROWS_PER_CORE)
    shards = x.reshape(N_CORES, ROWS_PER_CORE, FEAT)
    in_maps = [
        {"x": shards[i], "wt": wt, "bt": bt} for i in range(N_CORES)
    ]
    res = run_bass_kernel_spmd(
        nc, in_maps, core_ids=list(range(N_CORES)), trace=PROFILE
    )
    LAST_RESULTS = res
    return np.concatenate([r["y"] for r in res.results], axis=0)
